# revision 48
# baseline (speedup 1.0000x reference)
"""Multi-head attention (B=8, N=1024, D=1024, H=16, Dh=64) on 8 TRN2 NeuronCores.

Sharding: pure data-parallel over batch — core i computes batch element i
end-to-end; weights are replicated. No collectives.

bf16 end-to-end: inputs are cast to bf16 on the HOST (numpy) so DMA traffic
halves and no on-device staging/convert is needed; the device output is
bf16 too (upcast to f32 on the host). rel err ~6.1e-3 vs the fp32
reference (gate is 2e-2).

PE is the wall: 460k moving-rows at bf16 1 cyc/row = 191.6us at 2.4GHz
(qkv proj 196.6k, scores 131k, av 66.5k, out-proj 65.5k). The schedule
keeps PE dense; ACT (128 exp tiles ~133us) and DVE (~95us) ride under it.

DMA ring rule (sim cost model + BIR waits): a DMACopy<->DmaTransposeAnt
kind-switch in the global DMA rotation order waits for the full completion
of the DMA before it. So kinds are grouped into long runs: [q+k slab
copies] [all 16 xT transposes] [wv halves, bias] — two switches total.
Cross-ring tricks (ACT/Pool rings) measured worse or wrong; everything
stays on the SP ring.

Schedule:
  A: xT built by XBAR DMA-transpose straight from DRAM; slab-0 q/k land
     first and the two c0 projection chains interleave dt-by-dt so each
     arriving transpose feeds 854ns of matmul work against its ~625ns
     HWDGE cadence (no chase stalls, PE pstate stays warm). Three score
     pairs are emitted up front — wv's first column-half (split DMA)
     lands ~22.6us and the v-projection c0 chunks chase it. v is stored
     [n, 16*(64+1)] with a ones column per head so the av matmul emits
     softmax denominators for free.
  B+D per head pair hp: proj_hp, scores_hp, av_{hp-1}. Scores feed ACT
     before av consumes the PREVIOUS pair's attn tiles (attn pool bufs=2),
     so ACT never starves at pair boundaries. av runs in [i, dv|den]
     layout — moving dim 65, denominator in the same partition as its
     queries so normalize is a per-partition tensor_scalar. mergedT is
     rebuilt per pair by SBUF->SBUF XBAR DMA-transposes (no PE/DVE).
  E spread across the pair pipeline: out-proj at-pair (0,1) accumulates
     during pair 4 (bias folded), (2,3) during pair 5, (4,5) during pair
     7 (mergedT[a] exists only after pair a+1, so (a,a+1) runs at pair
     a+3 earliest). Chains go through the psqk psum pool and accumulate
     into bf16 SBUF accumulators with ping-pong buffers (accp bufs=2 —
     DVE in/out aliasing is illegal; psum must be tensor_tensor's in0).
     E-chains interleave 1:1 with the av blocks so PE never head-of-line
     blocks on psum rotation. w_out prefetched in ONE batched DMA during
     pair 3 into the tile wv used in A.
  Tail: av7 through its own double-buffered psum pools (psf 4 banks +
     psav7A/B 2+2) so the 16 blocks run back-to-back with no DVE
     round-trip exposure; mergedT[7] stripe transposes chase the
     normalizes; then per nt: the (6,7) psum chain + one DVE
     tensor_tensor (psum + acc -> stage). Stages are carved from the two
     dead [P,4096] slab buffers, so the out-DMAs batch into [512,1024] /
     [256,1024] transfers.

TimelineSim: 217.7us (baseline at session start: 225.0us).
"""

import sys

sys.path.insert(0, "/opt/trn_rl_repo")

import numpy as np

B, N, DIM = 8, 1024, 1024
HEADS, DH = 16, 64
P = 128
T = N // P  # 8 tiles per 1024 dim
SCALE = DH**-0.5

_CACHE = {}


def _emit_body(nc, tc, tile, mybir, x_ext, wqkv_ext, wout_ext, bout_ext, out_ext, sfx):
    """Emit one full attention pass reading x_ext (bf16), writing out_ext."""
    F32 = mybir.dt.float32
    BF16 = mybir.dt.bfloat16
    Exp = mybir.ActivationFunctionType.Exp
    mult = mybir.AluOpType.mult
    add = mybir.AluOpType.add

    CHUNKS = [(0, 512), (512, 512)]  # matmul moving size is ISA-capped at 512
    ODT = out_ext.dtype

    with (
        tc.tile_pool(name=f"const{sfx}", bufs=1) as const,
        tc.tile_pool(name=f"merged{sfx}", bufs=1) as merged_pool,
        tc.tile_pool(name=f"xt{sfx}", bufs=1) as xt_pool,
        tc.tile_pool(name=f"vp{sfx}", bufs=1) as v_pool,
        tc.tile_pool(name=f"qk{sfx}", bufs=1) as qkp,
        tc.tile_pool(name=f"wqk{sfx}", bufs=2) as wqkp,
        tc.tile_pool(name=f"attn{sfx}", bufs=2) as attnp,
        tc.tile_pool(name=f"wo{sfx}", bufs=1) as woutp,
        tc.tile_pool(name=f"outp{sfx}", bufs=1) as outp,
        tc.tile_pool(name=f"small{sfx}", bufs=2) as small,
        tc.tile_pool(name=f"rcp{sfx}", bufs=6) as rcp,
        tc.tile_pool(name=f"accu{sfx}", bufs=2) as accp,
    ):
        # warm the ACT exp table set at t=0
        wsrc = const.tile([1, 1], F32, tag="wsrc", name=f"wsrc{sfx}")
        nc.gpsimd.memset(wsrc[:], 0.0)
        warm = const.tile([1, 1], F32, tag="warm", name=f"warm{sfx}")
        nc.scalar.activation(warm[:], wsrc[:], Exp)


        mergedT = [
            merged_pool.tile([P, N], BF16, tag=f"m{a}", name=f"m{a}{sfx}")
            for a in range(T)
        ]
        merged_i = [
            merged_pool.tile([P, N], BF16, tag=f"mi{t}", name=f"mi{t}{sfx}")
            for t in range(T)
        ]
        xT = [
            xt_pool.tile([P, N], BF16, tag=f"xT{t}", name=f"xT{t}{sfx}")
            for t in range(T)
        ]
        # flat [128, 16*65]; head h's [v | ones] block is cols h*65..h*65+65
        v = [
            v_pool.tile([P, HEADS * (DH + 1)], BF16, tag=f"v{t}", name=f"v{t}{sfx}")
            for t in range(T)
        ]

        def load_slab(k):
            # both q/k halves of quarter-slab k in ONE batched DMA:
            # DRAM {q cols k*256:+256} ∪ {k cols DIM+k*256:+256} ->
            # SBUF [128, 2*8dt*256]. Same total descriptor time as two
            # DMAs, one ring slot (per-DMA SEQ/HWDGE overhead is ~0.6us).
            w_sb = wqkp.tile([P, 2 * T * 256], BF16, tag="ws", name=f"ws{sfx}")
            for s in range(2):
                base = k * 256 + s * DIM
                nc.sync.dma_start(
                    w_sb[:, s * T * 256 : (s + 1) * T * 256].rearrange(
                        "p (dt c) -> p dt c", c=256
                    ),
                    wqkv_ext[:, base : base + 256].rearrange(
                        "(dt p) c -> p dt c", p=P
                    ),
                )
            out = []
            for s in range(2):
                base = s * T * 256
                out.append(
                    [w_sb[:, base + dt * 256 : base + (dt + 1) * 256] for dt in range(T)]
                )
            return out

        def proj_half(w_tiles, col, dst, c, w, psum_pool, tag):
            """One 512-wide projection chunk: dst[:, c:c+w] = (w.T @ xT)[…]."""
            ps = psum_pool.tile([P, 512], F32, tag=tag, name=f"pp{sfx}")
            for dt in range(T):
                nc.tensor.matmul(
                    ps[:],
                    w_tiles[dt][:, col * P : (col + 1) * P],
                    xT[dt][:, c : c + w],
                    start=(dt == 0),
                    stop=(dt == T - 1),
                )
            nc.vector.tensor_copy(dst[:, c : c + w], ps[:])

        def emit_score_pair(jt, q_sb, k_sb, pss):
            """scores + exp for j-tile jt, both heads (adjacent row-groups)."""
            out = []
            for sub in range(2):
                ro = sub * DH
                ps_s = pss.tile([P, N], F32, tag="pss", name=f"pss{sfx}")
                for c, w in CHUNKS:
                    nc.tensor.matmul(
                        ps_s[:, c : c + w],
                        k_sb[ro : ro + DH, jt * P : (jt + 1) * P],
                        q_sb[ro : ro + DH, c : c + w],
                        start=True,
                        stop=True,
                    )
                at_sb = attnp.tile(
                    [P, N], BF16, tag=f"at{jt}_{sub}", name=f"at{jt}_{sub}{sfx}"
                )
                nc.scalar.activation(at_sb[:], ps_s[:], Exp, scale=SCALE)
                out.append(at_sb)
            return out

        def emit_av_block(hp, attn_tiles, sub, it, psav, wide=False):
            """attn @ v for head 2*hp+sub, i-block it, in [i, dv] layout:
            out[i, dv|den] = sum_j attnT[j,i].T @ [v|1][j,dv] — moving dim 65
            at bf16 full rate (8.3k cycles/pair vs 16.4k the other way).
            The softmax denominator lands in column 64 of the SAME partition
            as its queries, so normalize is a native per-partition
            tensor_scalar — no partition_broadcast needed."""
            h = 2 * hp + sub
            if wide:
                ps_b = psav.tile([P, 512], F32, tag="psqk", name=f"psb{sfx}")[
                    :, 0 : DH + 1
                ]
            else:
                ps_b = psav.tile([P, DH + 1], F32, tag="psb", name=f"psb{sfx}")
            for jt in range(T):
                nc.tensor.matmul(
                    ps_b,
                    attn_tiles[sub][jt][:, it * P : (it + 1) * P],
                    v[jt][:, h * (DH + 1) : (h + 1) * (DH + 1)],
                    start=(jt == 0),
                    stop=(jt == T - 1),
                )
            rc = rcp.tile([P, 1], F32, tag="rc", name=f"rc{sfx}")
            nc.vector.reciprocal(rc[:], ps_b[:, DH : DH + 1])
            nc.vector.tensor_scalar(
                merged_i[it][:, h * DH : (h + 1) * DH],
                ps_b[:, 0:DH],
                rc[:, 0:1],
                None,
                mult,
            )

        def emit_av_pair(hp, attn_tiles, psavA, psavB, extra=()):
            # `extra`: thunks (e.g. out-proj E-chains) interleaved 1:1 with
            # the av blocks so PE always has a ready instruction while the
            # psqk psum tiles rotate through their DVE readers.
            ex = list(extra)
            xi = 0
            for it in range(T):
                emit_av_block(hp, attn_tiles, 0, it, psavA)
                if xi < len(ex):
                    ex[xi]()
                    xi += 1
                emit_av_block(hp, attn_tiles, 1, it, psavB)
                if xi < len(ex):
                    ex[xi]()
                    xi += 1
            while xi < len(ex):
                ex[xi]()
                xi += 1

        def emit_mergedT(hp):
            # mergedT[hp] via SBUF->SBUF XBAR DMA transpose (no PE/DVE)
            for it in range(T):
                nc.sync.dma_start_transpose(
                    mergedT[hp][:, it * P : (it + 1) * P],
                    merged_i[it][:, hp * P : (hp + 1) * P],
                )

        # ---- Phase A: XBAR-transposed x + pair-0 proj/scores + C ----
        with (
            tc.tile_pool(name=f"psv{sfx}", bufs=2, space="PSUM") as psv,
            tc.tile_pool(name=f"pssA{sfx}", bufs=3, space="PSUM") as pssA,
        ):
            # DMA ring order: a copy<->transpose kind-switch waits for the
            # full completion of the DMA ahead of it (XBAR mode switch), so
            # group kinds into long runs: [q-slab, k-slab] copies first,
            # then all 16 xT transposes, then wv+bias. Costs one switch
            # before t1 (waits k-slab, ~5us) and one before wv (waits t2).
            slabs = {0: load_slab(0)}
            for dt in range(T):
                nc.sync.dma_start_transpose(
                    xT[dt][:, 0:512], x_ext[0:512, dt * P : (dt + 1) * P]
                )
            for dt in range(T):
                nc.sync.dma_start_transpose(
                    xT[dt][:, 512:1024], x_ext[512:1024, dt * P : (dt + 1) * P]
                )
            # wv in TWO batched column-half DMAs into the tile w_out will
            # reuse later: emit_C's c0 chunks only need cols 0:512, so the
            # first half unblocks the v-projection ~3us earlier
            wv_sb = woutp.tile([P, T * DIM], BF16, tag="wo", name=f"wv{sfx}")
            for c, w in CHUNKS:
                nc.sync.dma_start(
                    wv_sb[:].rearrange("p (dt c) -> p dt c", c=DIM)[:, :, c : c + w],
                    wqkv_ext[:, 2 * DIM + c : 2 * DIM + c + w].rearrange(
                        "(dt p) c -> p dt c", p=P
                    ),
                )
            wv = [wv_sb[:, dt * DIM : (dt + 1) * DIM] for dt in range(T)]
            b_sb = small.tile([1, DIM], BF16, tag="b_sb", name=f"b_sb{sfx}")
            nc.sync.dma_start(b_sb[:], bout_ext[:])
            b_bcast = outp.tile([P, DIM], BF16, tag="b_bcast", name=f"b_bcast{sfx}")
            nc.gpsimd.partition_broadcast(b_bcast[:], b_sb[:])

            def emit_C(k):
                # v-projection for x row k
                nc.gpsimd.memset(v[k][:], 1.0)
                for c, w in CHUNKS:
                    ps = psv.tile([P, 512], F32, tag="psv", name=f"psv{sfx}")
                    for dt in range(T):
                        nc.tensor.matmul(
                            ps[:],
                            xT[dt][:, k * P : (k + 1) * P],
                            wv[dt][:, c : c + w],
                            start=(dt == 0),
                            stop=(dt == T - 1),
                        )
                    nc.vector.tensor_copy(
                        v[k][:].rearrange("p (h c) -> p h c", c=DH + 1)[
                            :, (c // DH) : (c // DH) + 8, 0:DH
                        ],
                        ps[:].rearrange("p (h c) -> p h c", c=DH),
                    )

            # pair-0 q/k projection. The two c0 chains are interleaved
            # dt-by-dt so each landing xT transpose feeds 854ns of matmul
            # work against its ~625ns HWDGE cadence — no chase stalls, and
            # the PE pstate ramp stays warm.
            q_sb = qkp.tile([P, N], BF16, tag="q0", name=f"q0{sfx}")
            k_sb = qkp.tile([P, N], BF16, tag="k0", name=f"k0{sfx}")
            wq0, wk0 = slabs[0]

            def proj_pair_interleaved(c, w):
                psA = psv.tile([P, 512], F32, tag="psv", name=f"ppA{sfx}")
                psB = psv.tile([P, 512], F32, tag="psv", name=f"ppB{sfx}")
                for dt in range(T):
                    nc.tensor.matmul(
                        psA[:],
                        wq0[dt][:, 0:P],
                        xT[dt][:, c : c + w],
                        start=(dt == 0),
                        stop=(dt == T - 1),
                    )
                    nc.tensor.matmul(
                        psB[:],
                        wk0[dt][:, 0:P],
                        xT[dt][:, c : c + w],
                        start=(dt == 0),
                        stop=(dt == T - 1),
                    )
                nc.vector.tensor_copy(q_sb[:, c : c + w], psA[:])
                nc.vector.tensor_copy(k_sb[:, c : c + w], psB[:])

            proj_pair_interleaved(0, 512)
            proj_pair_interleaved(512, 512)

            # pair-0 scores interleaved with the v-projection: ACT starts
            # exp'ing early while the v matmuls keep PE busy. Two score
            # pairs go first — wv lands a couple of us after the
            # projections finish, so C(0) would otherwise stall PE.
            attn0 = [[], []]
            UPFRONT = 3
            for jt in range(UPFRONT):
                s0, s1 = emit_score_pair(jt, q_sb, k_sb, pssA)
                attn0[0].append(s0)
                attn0[1].append(s1)
            for jt in range(UPFRONT, T):
                s0, s1 = emit_score_pair(jt, q_sb, k_sb, pssA)
                attn0[0].append(s0)
                attn0[1].append(s1)
                emit_C(jt - UPFRONT)
            for k in range(T - UPFRONT, T):
                emit_C(k)

        # ---- Phases B+D pipelined per head pair; phase E spread across
        # pairs 4..6: out-proj at-pair (0,1) accumulates during pair 4
        # (bias folded in), (2,3) during pair 5, (4,5) during pair 6, each
        # as psum chains through the psqk pool + an in-place DVE add into
        # bf16 SBUF accumulators. The E-chains are interleaved 1:1 with the
        # av blocks so PE never head-of-line blocks on psum rotation. Only
        # av7 + the (6,7) chain + final combine remain in the tail. ----
        # acc ping-pong: each E-step reads the previous buffer and writes
        # the next (accp bufs=2 rotation) — DVE in/out aliasing is illegal.
        acc_t = [None] * T
        with (
            tc.tile_pool(name=f"psqk{sfx}", bufs=2, space="PSUM") as psqk,
            tc.tile_pool(name=f"pss{sfx}", bufs=2, space="PSUM") as pss,
            tc.tile_pool(name=f"psavA{sfx}", bufs=1, space="PSUM") as psavA,
            tc.tile_pool(name=f"psavB{sfx}", bufs=1, space="PSUM") as psavB,
        ):
            prev_attn = attn0
            wout_tiles = {}

            def e_chain(ats, nt, c, w, first):
                ps = psqk.tile([P, 512], F32, tag="psqk", name=f"pse{sfx}")
                for i, at in enumerate(ats):
                    nc.tensor.matmul(
                        ps[:],
                        mergedT[at][:, nt * P : (nt + 1) * P],
                        wout_tiles[at][:, c : c + w],
                        start=(i == 0),
                        stop=(i == len(ats) - 1),
                    )
                if c == 0:
                    # rotate to the write buffer for this step (both chunks
                    # of a step share one buffer; read side is the previous)
                    prev, cur = acc_t[nt], accp.tile(
                        [P, N], BF16, tag=f"acc{nt}", name=f"acc{nt}{sfx}"
                    )
                    acc_t[nt] = (prev, cur)
                prev, cur = acc_t[nt]
                if first:
                    nc.vector.tensor_tensor(
                        cur[:, c : c + w], ps[:], b_bcast[:, c : c + w], add
                    )
                else:
                    nc.vector.tensor_tensor(
                        cur[:, c : c + w], ps[:], prev[:, c : c + w], add
                    )
                if c == CHUNKS[-1][0]:
                    acc_t[nt] = cur

            def e_thunks(ats, first):
                return [
                    (lambda nt=nt, c=c, w=w: e_chain(ats, nt, c, w, first))
                    for nt in range(T)
                    for c, w in CHUNKS
                ]

            # mergedT[a] is emitted at the END of pair a+1, so step (a, a+1)
            # can run during pair a+3 at the earliest
            E_STEPS = {4: ((0, 1), True), 5: ((2, 3), False), 7: ((4, 5), False)}
            for hp in range(1, 8):
                if hp in (1, 3, 5):
                    slabs[(hp + 1) // 2] = load_slab((hp + 1) // 2)
                # w_out prefetch in one batched DMA (reuses the wv tile);
                # lands well before the first E-chain late in pair 4
                if hp == 3:
                    wo_sb = woutp.tile([P, T * DIM], BF16, tag="wo", name=f"wo{sfx}")
                    nc.sync.dma_start(
                        wo_sb[:].rearrange("p (dt c) -> p dt c", c=DIM),
                        wout_ext[:].rearrange("(dt p) c -> p dt c", p=P),
                    )
                    for at in range(T):
                        wout_tiles[at] = wo_sb[:, at * DIM : (at + 1) * DIM]
                wq, wk = slabs[hp // 2]
                q_sb = qkp.tile([P, N], BF16, tag=f"q{hp % 2}", name=f"q{hp}{sfx}")
                k_sb = qkp.tile([P, N], BF16, tag=f"k{hp % 2}", name=f"k{hp}{sfx}")
                col = hp % 2
                proj_half(wq, col, q_sb, 0, 512, psqk, "psqk")
                proj_half(wq, col, q_sb, 512, 512, psqk, "psqk")
                proj_half(wk, col, k_sb, 0, 512, psqk, "psqk")
                proj_half(wk, col, k_sb, 512, 512, psqk, "psqk")
                attn_tiles = [[], []]
                for jt in range(T):
                    s0, s1 = emit_score_pair(jt, q_sb, k_sb, pss)
                    attn_tiles[0].append(s0)
                    attn_tiles[1].append(s1)
                ats, first = E_STEPS.get(hp, (None, False))
                extra = e_thunks(ats, first) if ats is not None else ()
                emit_av_pair(hp - 1, prev_attn, psavA, psavB, extra=extra)
                emit_mergedT(hp - 1)
                prev_attn = attn_tiles

            # output staging carved from the two dead slab buffers (tag
            # "ws" x bufs=2), four [P, N] stages per buffer — so the
            # out-DMAs batch: one [512, 1024] DMA for nt 0-3, and two
            # [256, 1024] DMAs for (4,5) and (6,7) (split so the last
            # transfer is small and fires right after final(7))
            stage = []
            stage_buf = []
            for _ in range(2):
                big = wqkp.tile(
                    [P, 2 * T * 256], BF16, tag="ws", name=f"accw{sfx}"
                )
                for q in range(4):
                    stage.append(big[:, q * N : (q + 1) * N])
                stage_buf.append(big)

        # ---- Tail: av7 (own double-buffered psum pools so the 16 blocks
        # run back-to-back with no DVE round-trip exposure), mergedT[7]
        # stripe transposes chasing the normalizes, then per nt the (6,7)
        # psum chain + one DVE tensor_tensor (psum + acc -> stage). ----
        with (
            tc.tile_pool(name=f"psf{sfx}", bufs=2, space="PSUM") as psf,
            tc.tile_pool(name=f"psv7A{sfx}", bufs=2, space="PSUM") as psav7A,
            tc.tile_pool(name=f"psv7B{sfx}", bufs=2, space="PSUM") as psav7B,
        ):
            for it in range(T):
                emit_av_block(7, prev_attn, 0, it, psav7A)
                emit_av_block(7, prev_attn, 1, it, psav7B)
                nc.sync.dma_start_transpose(
                    mergedT[7][:, it * P : (it + 1) * P],
                    merged_i[it][:, 7 * P : 8 * P],
                )
            for nt in range(T):
                ps = psf.tile([P, DIM], F32, tag="psf", name=f"psf{sfx}")
                for i, at in enumerate((6, 7)):
                    lhsT = mergedT[at][:, nt * P : (nt + 1) * P]
                    for c, w in CHUNKS:
                        nc.tensor.matmul(
                            ps[:, c : c + w],
                            lhsT,
                            wout_tiles[at][:, c : c + w],
                            start=(i == 0),
                            stop=(i == 1),
                        )
                # acc fold + psum read on DVE (free in the tail: av7's
                # normalizes are done early thanks to the deep psum pools)
                nc.vector.tensor_tensor(stage[nt], ps[:], acc_t[nt][:], add)
                if nt == 3:
                    nc.sync.dma_start(
                        out_ext[0 : 4 * P, :].rearrange("(n p) c -> p n c", p=P),
                        stage_buf[0][:].rearrange("p (n c) -> p n c", c=N),
                    )
                elif nt in (5, 7):
                    nc.sync.dma_start(
                        out_ext[(nt - 1) * P : (nt + 1) * P, :].rearrange(
                            "(n p) c -> p n c", p=P
                        ),
                        stage_buf[1][:, (nt - 5) * N : (nt - 3) * N].rearrange(
                            "p (n c) -> p n c", c=N
                        ),
                    )

def _build(reps=1, variant=None):
    import concourse.tile as tile
    from concourse import bacc, mybir

    F32 = mybir.dt.float32
    BF16 = mybir.dt.bfloat16

    nc = bacc.Bacc("TRN2", target_bir_lowering=False, debug=False, num_devices=8)
    x_ext = nc.declare_dram_parameter("x", [N, DIM], BF16, isOutput=False)
    wqkv_ext = nc.declare_dram_parameter("w_qkv", [DIM, 3 * DIM], BF16, isOutput=False)
    wout_ext = nc.declare_dram_parameter("w_out", [DIM, DIM], BF16, isOutput=False)
    bout_ext = nc.declare_dram_parameter("b_out", [1, DIM], BF16, isOutput=False)
    out_ext = nc.declare_dram_parameter("out", [N, DIM], BF16, isOutput=True)
    bounce = [
        nc.dram_tensor(f"bounce{k}", [N, DIM], BF16) for k in range(max(0, reps - 1))
    ]

    with tile.TileContext(nc) as tc:
        for k in range(reps):
            src = x_ext if k == 0 else bounce[k - 1]
            dst = out_ext if k == reps - 1 else bounce[k]
            _emit_body(
                nc, tc, tile, mybir, src, wqkv_ext, wout_ext, bout_ext, dst, f"_{k}"
            )
    nc.compile()
    return nc


def _get_nc(reps=1, variant=None):
    key = ("nc", reps)
    if key not in _CACHE:
        _CACHE[key] = _build(reps)
    return _CACHE[key]


def run(inputs, trace=False, reps=1, variant=None):
    import ml_dtypes
    from concourse.bass_utils import run_bass_kernel_spmd

    BF = ml_dtypes.bfloat16
    nc = _get_nc(reps)
    x = np.ascontiguousarray(np.asarray(inputs["x"]).astype(BF))
    w_qkv = np.ascontiguousarray(np.asarray(inputs["w_qkv"]).astype(BF))
    w_out = np.ascontiguousarray(np.asarray(inputs["w_out"]).astype(BF))
    b_out = np.ascontiguousarray(np.asarray(inputs["b_out"]).astype(BF)).reshape(1, DIM)
    in_maps = [
        {"x": x[i], "w_qkv": w_qkv, "w_out": w_out, "b_out": b_out} for i in range(B)
    ]
    res = run_bass_kernel_spmd(nc, in_maps, core_ids=list(range(B)), trace=trace)
    out = np.stack([res.results[i]["out"] for i in range(B)]).astype(np.float32)
    return out, res


def kernel(**inputs) -> np.ndarray:
    out, _ = run(inputs)
    return out

